# revision 23
# baseline (speedup 1.0000x reference)
"""Multi-head attention (B=4, S=1024, H=1024, 16 heads) on 8 trn2 cores.

Sharding: 8 shards = (batch b in 0..3) x (head-half hf in 0..1).
Each core computes attention for 8 heads of one batch and a partial
output projection (row-parallel Wo); host sums the two partials per batch.

Per-core pipeline (matmuls in bf16, PSUM fp32, bf16 partial output):
  - V projection token-major with a ones column appended per head
    (row 64 of the attn@V psum then holds the softmax denominator);
    emitted in k-halves over m-pairs so matmuls trickle in as the
    quarter-split input DMAs land
  - the head section is one flat stream of 128 half-iterations
    (head, sk, n-half): logitsT via lhsT=KT tile (K=64 contraction)
    into a [128,512] psum, exp on ACT with the per-key bias fused
    (logits are O(+-9): fp32 exp needs no max-subtraction), and the
    attn@V matmul lagged LAG slots behind via a deferred-emission
    queue so the PE never waits on the exp stream (any PE gap would
    downclock the tensor engine 2x for the next 3us)
  - the next pair's QT/KT projection matmuls (heads 0-5) and the
    pair-0..2 output-projection PSUM groups (heads 6-7, evicted to a
    bf16 SBUF partial) are interleaved 1-2 matmuls per slot between
    the logits and attn@V matmuls, keeping the PE stream gapless
  - normalize via DVE reciprocal + gpsimd partition broadcast + DVE
    mul per n-half, emitted through the same deferred queue
  - tail: per output tile, an identity matmul injects the bf16
    pairs-0..2 partial into the pair-3 PSUM accumulation group (no
    vector adds, no at3 dependency, so it overlaps the last
    normalize); one ACT/DVE copy -> bf16 streams out per m-tile
  - input DMAs are consolidated into few large transfers, host
    pre-packs the slab layouts, and all triggers run on the SP queue
    so no compute queue stalls on the shared HWDGE unit
  - PSUM (8 banks): av 2x[65,1024]=4, lg 2x[128,512]=2, scratch
    2x[128,512]=2; the tail releases lg+scratch and reuses them
"""

import numpy as np
import ml_dtypes

import concourse.bass as bass
import concourse.tile as tile
from concourse import bacc, mybir
from concourse import bass_utils

F32 = mybir.dt.float32
BF16 = mybir.dt.bfloat16
EXP = mybir.ActivationFunctionType.Exp
COPY = mybir.ActivationFunctionType.Copy

S = 1024  # sequence length (tokens)
HID = 1024  # model hidden
DQ = 512  # per-core projected dim (8 heads x 64)
NHL = 8  # local heads per core
DH = 64  # head depth
NK = HID // 128  # 8 contraction tiles over hidden
P = 128
N_CORES = 8

GRAN = 512  # logits/exp tile width: 512 (fine slots) or 1024 (coarse slots)

_CACHED_NC = None


def build_program(unroll=1):
    nc = bacc.Bacc("TRN2", target_bir_lowering=False, debug=False)
    # host ships pre-packed slab layouts (see _prep_in_maps)
    xt = nc.dram_tensor("xt", [P, 2 * NK * 512], BF16, kind="ExternalInput").ap()
    yt = nc.dram_tensor("yt", [P, 2 * NK * 512], BF16, kind="ExternalInput").ap()
    wq = nc.dram_tensor("wq", [P, 4 * NK * P], BF16, kind="ExternalInput").ap()
    wk = nc.dram_tensor("wk", [P, 4 * NK * P], BF16, kind="ExternalInput").ap()
    wv = nc.dram_tensor("wv", [P, NK * DQ], BF16, kind="ExternalInput").ap()
    wo = nc.dram_tensor("wo", [P, 4 * HID], BF16, kind="ExternalInput").ap()
    biasd = nc.dram_tensor("biasd", [P, NK], F32, kind="ExternalInput").ap()
    onesd = nc.dram_tensor("onesd", [P, NHL], BF16, kind="ExternalInput").ap()
    identd = nc.dram_tensor("identd", [P, P], BF16, kind="ExternalInput").ap()
    out = nc.dram_tensor("out", [S, HID], BF16, kind="ExternalOutput").ap()

    with tile.TileContext(nc) as tc:
        for _ in range(unroll):
            emit_kernel(tc, out, xt, yt, wq, wk, wv, wo, biasd, onesd, identd)
    nc.compile()
    return nc


def emit_kernel(tc, out, xt, yt, wq, wk, wv, wo, biasd, onesd, identd):
    nc = tc.nc
    with (
        tc.tile_pool(name="inpool", bufs=1) as inpool,
        tc.tile_pool(name="qkv", bufs=1) as qkvpool,
        tc.tile_pool(name="atp", bufs=1) as atpool,
        tc.tile_pool(name="expp", bufs=4) as exppool,
        tc.tile_pool(name="smallp", bufs=2) as smallpool,
        tc.tile_pool(name="accp", bufs=1) as accpool,
        tc.tile_pool(name="outp", bufs=4) as outpool,
    ):
        # ---- input slabs (DMA'd in large consolidated transfers) ----
        wv_slab = inpool.tile([P, NK * DQ], BF16, tag="wv", name="wv_slab")
        yt_slab = inpool.tile([P, NK * S], BF16, tag="yt", name="yt_slab")
        xt_slab = inpool.tile([P, NK * S], BF16, tag="xt", name="xt_slab")
        wq_slab = inpool.tile([P, 4 * NK * P], BF16, tag="wq", name="wq_slab")
        wk_slab = inpool.tile([P, 4 * NK * P], BF16, tag="wk", name="wk_slab")
        wo_slab = inpool.tile([P, 4 * HID], BF16, tag="wo", name="wo_slab")
        bias_sb = inpool.tile([P, NK], F32, tag="bias", name="bias_sb")
        vones_sb = inpool.tile([P, NHL], BF16, tag="vones", name="vones_sb")
        ident_sb = inpool.tile([P, P], BF16, tag="ident", name="ident_sb")

        # issue order on SP = earliest-needed first
        yt3 = yt_slab[:].rearrange("p (k c) -> p k c", c=S)
        xt3 = xt_slab[:].rearrange("p (k c) -> p k c", c=S)
        for q in range(8):
            if q >= 2 and q % 2 == 1:
                continue  # eighth-split only the first quarter
            span = 1 if q < 2 else 2
            wvs = slice(q * (NK * DQ // 8), (q + span) * (NK * DQ // 8))
            nc.sync.dma_start(wv_slab[:, wvs], wv[:, wvs])
            nc.sync.dma_start(
                yt3[:, q : q + span, 0:512],
                yt[:, q * (NK * 512 // 8) : (q + span) * (NK * 512 // 8)],
            )
        nc.sync.dma_start(yt3[:, :, 512:1024], yt[:, NK * 512 : 2 * NK * 512])
        nc.sync.dma_start(bias_sb[:], biasd[:])
        nc.sync.dma_start(vones_sb[:], onesd[:])
        nc.sync.dma_start(ident_sb[:], identd[:])
        sl0 = slice(0, NK * P)
        nc.sync.dma_start(wq_slab[:, sl0], wq[:, sl0])
        nc.sync.dma_start(wk_slab[:, sl0], wk[:, sl0])
        nc.sync.dma_start(xt3[:, :, 0:512], xt[:, 0 : NK * 512])
        nc.sync.dma_start(xt3[:, :, 512:1024], xt[:, NK * 512 : 2 * NK * 512])
        for pair in range(1, 4):
            sl = slice(pair * NK * P, (pair + 1) * NK * P)
            nc.sync.dma_start(wq_slab[:, sl], wq[:, sl])
            nc.sync.dma_start(wk_slab[:, sl], wk[:, sl])
        for pair in range(4):
            sl = slice(pair * HID, (pair + 1) * HID)
            nc.sync.dma_start(wo_slab[:, sl], wo[:, sl])

        def wv_k(k):
            return wv_slab[:, k * DQ : (k + 1) * DQ]

        def yt_k(k):
            return yt_slab[:, k * S : (k + 1) * S]

        def xt_k(k):
            return xt_slab[:, k * S : (k + 1) * S]

        def wqk_pk(slab, pair, k):
            base = pair * NK * P + k * P
            return slab[:, base : base + P]

        def wo_p(pair):
            return wo_slab[:, pair * HID : (pair + 1) * HID]

        # ---- persistent slabs ----
        qt_sb = [
            qkvpool.tile([P, S], BF16, tag=f"qt{m}", name=f"qt{m}") for m in range(4)
        ]
        kt_sb = [
            qkvpool.tile([P, S], BF16, tag=f"kt{m}", name=f"kt{m}") for m in range(4)
        ]
        v_sb = [
            qkvpool.tile([P, NHL * (DH + 1)], BF16, tag=f"v{m}", name=f"v{m}")
            for m in range(8)
        ]
        at_sb = [
            atpool.tile([P, S], BF16, tag=f"at{m}", name=f"at{m}") for m in range(4)
        ]
        acc_sb = [
            accpool.tile([P, HID], BF16, tag=f"acc{m}", name=f"acc{m}")
            for m in range(8)
        ]

        # PSUM (8 banks): GRAN=512 -> lg 2x[128,512]=2, av 2x[65,1024]=4,
        # sc 2x[128,512]=2; GRAN=1024 -> lg 2x[128,1024]=4, av 1x=2, sc 2.
        # (av first: the tail releases lg+sc but keeps av, so the tail pool
        # lands on banks whose last readers finished early)
        pp_av = tc.alloc_tile_pool(
            name="pp_av", bufs=2 if GRAN == 512 else 1, space="PSUM"
        )
        pp_lg = tc.alloc_tile_pool(name="pp_lg", bufs=2, space="PSUM")
        pp_sc = tc.alloc_tile_pool(name="pp_sc", bufs=2, space="PSUM")

        # ---- V projection (token-major, ones columns appended); emitted in
        # k-halves over m-pairs so matmuls trickle in as DMA quarters land ----
        for mp in range(0, 8, 2):
            pss = {}
            for kk in range(2):
                for m in (mp, mp + 1):
                    if kk == 0:
                        pss[m] = pp_sc.tile([P, DQ], F32, tag="sc", name="sc")
                    for k in range(kk * NK // 2, (kk + 1) * NK // 2):
                        nc.tensor.matmul(
                            pss[m][:],
                            yt_k(k)[:, m * P : (m + 1) * P],
                            wv_k(k),
                            start=(k == 0),
                            stop=(k == NK - 1),
                        )
            for m in (mp, mp + 1):
                dst3 = v_sb[m][:].rearrange("p (h c) -> p h c", c=DH + 1)
                src3 = pss[m][:].rearrange("p (h c) -> p h c", c=DH)
                nc.vector.tensor_copy(dst3[:, :, 0:DH], src3[:, :, :])
                nc.vector.tensor_copy(
                    dst3[:, :, DH : DH + 1],
                    vones_sb[:].rearrange("p (a b) -> p a b", b=1),
                )

        # ---- QT/KT projection for one pair as 32 emit-chunks of 1 matmul
        # (the last chunk of each psum tile appends the DVE eviction) ----
        def proj_chunks(pair):
            chunks = []
            for w_slab, src_k, dst in (
                (wq_slab, xt_k, qt_sb),
                (wk_slab, yt_k, kt_sb),
            ):
                for n in range(2):
                    ps_box = [None]

                    def mm(k, w_slab=w_slab, src_k=src_k, dst=dst, n=n, ps_box=ps_box):
                        if k == 0:
                            ps_box[0] = pp_sc.tile([P, 512], F32, tag="sc", name="sc")
                        nc.tensor.matmul(
                            ps_box[0][:],
                            wqk_pk(w_slab, pair, k),
                            src_k(k)[:, n * 512 : (n + 1) * 512],
                            start=(k == 0),
                            stop=(k == NK - 1),
                        )
                        if k == NK - 1:
                            nc.vector.tensor_copy(
                                dst[pair][:, n * 512 : (n + 1) * 512], ps_box[0][:]
                            )

                    for k in range(NK):
                        chunks.append(lambda k=k, mm=mm: mm(k))
            return chunks

        # ---- pairs 0-2 of the output projection: one PSUM accumulation
        # group per (m, n) tile, evicted to fp32 SBUF partials; two chunks
        # per tile ----
        def wo012_chunks():
            # last RESERVE tiles are held back to run during the final
            # head's normalize; their evictions go to the then-idle ACT
            chunks = []
            for m in range(8):
                for n in range(2):
                    ps_box = [None]
                    act_evict = 2 * m + n >= 16 - WO_RESERVE

                    def part1(m=m, n=n, ps_box=ps_box):
                        ps_box[0] = pp_sc.tile([P, 512], F32, tag="sc", name="sc")
                        for pair in range(2):
                            nc.tensor.matmul(
                                ps_box[0][:],
                                at_sb[pair][:, m * P : (m + 1) * P],
                                wo_p(pair)[:, n * 512 : (n + 1) * 512],
                                start=(pair == 0),
                                stop=False,
                            )

                    def part2(m=m, n=n, ps_box=ps_box, act_evict=act_evict):
                        nc.tensor.matmul(
                            ps_box[0][:],
                            at_sb[2][:, m * P : (m + 1) * P],
                            wo_p(2)[:, n * 512 : (n + 1) * 512],
                            start=False,
                            stop=True,
                        )
                        dst = acc_sb[m][:, n * 512 : (n + 1) * 512]
                        if act_evict:
                            nc.scalar.activation(dst, ps_box[0][:], COPY)
                        else:
                            nc.vector.tensor_copy(dst, ps_box[0][:])

                    chunks.append(part1)
                    chunks.append(part2)
            return chunks

        # ---- head section: one flat stream of 128 half-iterations
        # (head, sk, n). attn@V matmuls lag by LAG slots via a deferred
        # queue so they never make the PE wait on the exp stream. ----
        LAG = 3 if GRAN == 512 else 2
        WO_RESERVE = 2
        pending = {}
        gctr = [0]

        def emit_head(h, extras, delay=0):
            pair, hi = divmod(h, 2)
            base = hi * DH
            av = pp_av.tile([DH + 1, S], F32, tag="av", name="av")
            ei = 0
            nslots = 2 * NK if GRAN == 512 else NK
            for j in range(nslots):
                g = gctr[0]
                gctr[0] += 1
                if GRAN == 512:
                    sk, n = divmod(j, 2)
                    nhs = [n]
                else:
                    sk, nhs = j, [0, 1]
                lg = pp_lg.tile([P, GRAN], F32, tag="lg", name="lg")
                for li, n in enumerate(nhs):
                    nc.tensor.matmul(
                        lg[:, li * 512 : (li + 1) * 512],
                        kt_sb[pair][base : base + DH, sk * P : (sk + 1) * P],
                        qt_sb[pair][base : base + DH, n * 512 : (n + 1) * 512],
                        start=True,
                        stop=True,
                    )
                e = exppool.tile([P, GRAN], BF16, tag="exp", name="exp")
                nc.scalar.activation(e[:], lg[:], EXP, bias=bias_sb[:, sk : sk + 1])
                # lagged attn@V / normalize closures first: extras of the
                # next phase may read what the trailing normalizes write
                for fn in pending.pop(g, []):
                    fn()
                # interleaved PE work runs while ACT streams the exp
                if j >= delay:
                    take = (len(extras) - ei + (nslots - 1 - j)) // (nslots - j - (delay - j if j < delay else 0))
                    for _ in range(take):
                        extras[ei]()
                        ei += 1

                def av_mm(sk=sk, nhs=nhs, e=e, av=av, h=h):
                    for li, n in enumerate(nhs):
                        nc.tensor.matmul(
                            av[:, n * 512 : (n + 1) * 512],
                            v_sb[sk][:, h * (DH + 1) : (h + 1) * (DH + 1)],
                            e[:, li * 512 : (li + 1) * 512],
                            start=(sk == 0),
                            stop=(sk == NK - 1),
                        )

                pending.setdefault(g + LAG, []).append(av_mm)
                if sk == NK - 1:
                    # normalize per n-half right after the last attn@V
                    def norm(n, av=av, pair=pair, base=base):
                        cs = slice(n * 512, (n + 1) * 512)
                        rc = smallpool.tile([1, S], F32, tag="rc", name="rc")
                        nc.vector.reciprocal(rc[:, cs], av[DH : DH + 1, cs])
                        bc_sb = smallpool.tile([DH, S], F32, tag="bcsb", name="bcsb")
                        nc.gpsimd.partition_broadcast(bc_sb[:, cs], rc[:, cs])
                        nc.vector.tensor_mul(
                            at_sb[pair][base : base + DH, cs],
                            av[0:DH, cs],
                            bc_sb[:, cs],
                        )

                    for nn_ in nhs:
                        pending.setdefault(g + LAG, []).append(
                            lambda norm=norm, nn_=nn_: norm(n=nn_)
                        )
            assert ei == len(extras)

        # proj for pair 0 runs standalone (DMA-gated region anyway)
        for ch in proj_chunks(0):
            ch()
        # heads 0..5 carry the next pair's projections; 6..7 carry the
        # pair-0..2 output projection groups
        for pair in range(3):
            nxt = proj_chunks(pair + 1)
            emit_head(2 * pair, nxt[:16])
            emit_head(2 * pair + 1, nxt[16:])
        wo012 = wo012_chunks()
        nres = 2 * WO_RESERVE
        emit_head(6, wo012[:16], delay=3)
        emit_head(7, wo012[16 : 32 - nres])
        # flush trailing lagged attn@V + normalize closures, interleaving the
        # reserved wo012 chunks so the PE stays busy through the normalize
        reserved = wo012[32 - nres :]
        flush = []
        for g in sorted(pending.keys()):
            flush.extend(pending.pop(g))
        fi = ri = 0
        while fi < len(flush) or ri < len(reserved):
            if fi < len(flush):
                flush[fi]()
                fi += 1
            if ri < len(reserved):
                reserved[ri]()
                ri += 1

        # ---- tail: per m-tile PSUM group = identity matmul injecting the
        # bf16 pairs-0..2 partial (no at3 dependency -> runs during the last
        # normalize) + the pair-3 matmul; one ACT/DVE copy -> bf16 streams
        # out. No vector adds. ----
        pp_sc.release()
        pp_lg.release()
        pp_tail = tc.alloc_tile_pool(name="pp_tail", bufs=4, space="PSUM")
        WARM = 4
        units = [(m, n) for m in range(8) for n in range(2)]
        tail_ps = {}
        ob_tiles = {}

        def emit_ident(u):
            m, n = units[u]
            ps = pp_tail.tile([P, 512], F32, tag="tl", name="tl")
            tail_ps[u] = ps
            nc.tensor.matmul(
                ps[:],
                ident_sb[:],
                acc_sb[m][:, n * 512 : (n + 1) * 512],
                start=True,
                stop=False,
            )

        for u in range(WARM):
            emit_ident(u)
        for u in range(16):
            m, n = units[u]
            ps = tail_ps[u]
            nc.tensor.matmul(
                ps[:],
                at_sb[3][:, m * P : (m + 1) * P],
                wo_p(3)[:, n * 512 : (n + 1) * 512],
                start=False,
                stop=True,
            )
            if u + WARM < 16:
                emit_ident(u + WARM)
            if n == 0:
                ob_tiles[m] = outpool.tile([P, HID], BF16, tag="ob", name="ob")
            ob = ob_tiles[m]
            dst = ob[:, n * 512 : (n + 1) * 512]
            if u % 2 == 1:
                nc.vector.tensor_copy(dst, ps[:])
            else:
                nc.scalar.activation(dst, ps[:], COPY)
            if n == 1:
                nc.sync.dma_start(out[m * P : (m + 1) * P, :], ob[:])
        pp_tail.release()
        pp_av.release()


def _prep_in_maps(x, y, bias, Wq, Wk, Wv, Wo):
    x = np.asarray(x, dtype=np.float32)
    y = np.asarray(y, dtype=np.float32)
    bias = np.asarray(bias, dtype=np.float32)
    Wq = np.asarray(Wq, dtype=np.float32)
    Wk = np.asarray(Wk, dtype=np.float32)
    Wv = np.asarray(Wv, dtype=np.float32)
    Wo = np.asarray(Wo, dtype=np.float32)
    scale = 1.0 / np.sqrt(DH)
    dt = ml_dtypes.bfloat16

    def act_slab(a):
        # activation a [S, HID] -> slab halves layout [128, 2*NK*512]:
        # [:, h*NK*512 + k*512 + c] = a.T[k*128+p, h*512+c]
        at = a.T.reshape(NK, P, 2, 512)  # [k, p, h, c]
        return np.ascontiguousarray(at.transpose(1, 2, 0, 3).reshape(P, 2 * NK * 512))

    def w_pair_slab(w):
        # weights [1024, 512] -> pair-major slab [128, (pair k c128)]
        wr = w.reshape(NK, P, 4, P)  # [k, p, pair, c]
        return np.ascontiguousarray(wr.transpose(1, 2, 0, 3).reshape(P, 4 * NK * P))

    def wv_slab(w):
        # weights [1024, 512] -> k-major slab [128, (k c512)]
        wr = w.reshape(NK, P, DQ)
        return np.ascontiguousarray(wr.transpose(1, 0, 2).reshape(P, NK * DQ))

    def wo_slab(w):
        # [512, 1024] -> pair-major slab [128, (pair c1024)]
        wr = w.reshape(4, P, HID)
        return np.ascontiguousarray(wr.transpose(1, 0, 2).reshape(P, 4 * HID))

    in_maps = []
    for c in range(N_CORES):
        b, hf = divmod(c, 2)
        cols = slice(hf * DQ, (hf + 1) * DQ)
        in_maps.append(
            {
                "xt": act_slab(x[b]).astype(dt),
                "yt": act_slab(y[b]).astype(dt),
                "wq": w_pair_slab(Wq[:, cols] * scale).astype(dt),
                "wk": w_pair_slab(Wk[:, cols]).astype(dt),
                "wv": wv_slab(Wv[:, cols]).astype(dt),
                "wo": wo_slab(Wo[cols, :]).astype(dt),
                "biasd": np.ascontiguousarray(bias[b, 0, 0].reshape(NK, P).T),
                "onesd": np.ones((P, NHL), dtype=dt),
                "identd": np.eye(P, dtype=dt),
            }
        )
    return in_maps


def get_program():
    global _CACHED_NC
    if _CACHED_NC is None:
        _CACHED_NC = build_program()
    return _CACHED_NC


def kernel(x, y, bias, Wq, Wk, Wv, Wo):
    nc = get_program()
    in_maps = _prep_in_maps(x, y, bias, Wq, Wk, Wv, Wo)
    res = bass_utils.run_bass_kernel_spmd(nc, in_maps, core_ids=list(range(N_CORES)))
    B = 4
    out = np.empty((B, S, HID), dtype=np.float32)
    for b in range(B):
        out[b] = res.results[2 * b]["out"].astype(np.float32) + res.results[2 * b + 1][
            "out"
        ].astype(np.float32)
    return out


# revision 24
# speedup vs baseline: 2.9232x; 2.9232x over previous
"""Multi-head attention (B=4, S=1024, H=1024, 16 heads) on 8 trn2 cores.

Sharding: 8 shards = (batch b in 0..3) x (head-half hf in 0..1).
Each core computes attention for 8 heads of one batch and a partial
output projection (row-parallel Wo); host sums the two partials per batch.

Per-core pipeline (matmuls in bf16, PSUM fp32, bf16 partial output):
  - V projection token-major with a ones column appended per head
    (row 64 of the attn@V psum then holds the softmax denominator);
    emitted in k-halves over m-pairs so matmuls trickle in as the
    quarter-split input DMAs land
  - the head section is one flat stream of 128 half-iterations
    (head, sk, n-half): logitsT via lhsT=KT tile (K=64 contraction)
    into a [128,512] psum, exp on ACT with the per-key bias fused
    (logits are O(+-9): fp32 exp needs no max-subtraction), and the
    attn@V matmul lagged LAG slots behind via a deferred-emission
    queue so the PE never waits on the exp stream (any PE gap would
    downclock the tensor engine 2x for the next 3us)
  - the next pair's QT/KT projection matmuls (heads 0-5) and the
    pair-0..2 output-projection PSUM groups (heads 6-7, evicted to a
    bf16 SBUF partial) are interleaved 1-2 matmuls per slot between
    the logits and attn@V matmuls, keeping the PE stream gapless
  - normalize via DVE reciprocal + gpsimd partition broadcast + DVE
    mul per n-half, emitted through the same deferred queue
  - tail: per output tile, an identity matmul injects the bf16
    pairs-0..2 partial into the pair-3 PSUM accumulation group (no
    vector adds, no at3 dependency, so it overlaps the last
    normalize); one ACT/DVE copy -> bf16 streams out per m-tile
  - input DMAs are consolidated into few large transfers, host
    pre-packs the slab layouts, and all triggers run on the SP queue
    so no compute queue stalls on the shared HWDGE unit
  - PSUM (8 banks): av 2x[65,1024]=4, lg 2x[128,512]=2, scratch
    2x[128,512]=2; the tail releases lg+scratch and reuses them
"""

import numpy as np
import ml_dtypes

import concourse.bass as bass
import concourse.tile as tile
from concourse import bacc, mybir
from concourse import bass_utils

F32 = mybir.dt.float32
BF16 = mybir.dt.bfloat16
EXP = mybir.ActivationFunctionType.Exp
COPY = mybir.ActivationFunctionType.Copy

S = 1024  # sequence length (tokens)
HID = 1024  # model hidden
DQ = 512  # per-core projected dim (8 heads x 64)
NHL = 8  # local heads per core
DH = 64  # head depth
NK = HID // 128  # 8 contraction tiles over hidden
P = 128
N_CORES = 8

GRAN = 512  # logits/exp tile width: 512 (fine slots) or 1024 (coarse slots)

_CACHED_NC = None


def build_program(unroll=1):
    nc = bacc.Bacc("TRN2", target_bir_lowering=False, debug=False)
    # host ships pre-packed slab layouts (see _prep_in_maps)
    xt = nc.dram_tensor("xt", [P, 2 * NK * 512], BF16, kind="ExternalInput").ap()
    yt = nc.dram_tensor("yt", [P, 2 * NK * 512], BF16, kind="ExternalInput").ap()
    wq = nc.dram_tensor("wq", [P, 4 * NK * P], BF16, kind="ExternalInput").ap()
    wk = nc.dram_tensor("wk", [P, 4 * NK * P], BF16, kind="ExternalInput").ap()
    wv = nc.dram_tensor("wv", [P, NK * DQ], BF16, kind="ExternalInput").ap()
    wo = nc.dram_tensor("wo", [P, 4 * HID], BF16, kind="ExternalInput").ap()
    biasd = nc.dram_tensor("biasd", [P, NK], F32, kind="ExternalInput").ap()
    onesd = nc.dram_tensor("onesd", [P, NHL], BF16, kind="ExternalInput").ap()
    identd = nc.dram_tensor("identd", [P, P], BF16, kind="ExternalInput").ap()
    out = nc.dram_tensor("out", [S, HID], BF16, kind="ExternalOutput").ap()

    with tile.TileContext(nc) as tc:
        for _ in range(unroll):
            emit_kernel(tc, out, xt, yt, wq, wk, wv, wo, biasd, onesd, identd)
    nc.compile()
    return nc


def emit_kernel(tc, out, xt, yt, wq, wk, wv, wo, biasd, onesd, identd):
    nc = tc.nc
    with (
        tc.tile_pool(name="inpool", bufs=1) as inpool,
        tc.tile_pool(name="qkv", bufs=1) as qkvpool,
        tc.tile_pool(name="atp", bufs=1) as atpool,
        tc.tile_pool(name="expp", bufs=6) as exppool,
        tc.tile_pool(name="smallp", bufs=2) as smallpool,
        tc.tile_pool(name="accp", bufs=1) as accpool,
        tc.tile_pool(name="outp", bufs=4) as outpool,
    ):
        # ---- input slabs (DMA'd in large consolidated transfers) ----
        wv_slab = inpool.tile([P, NK * DQ], BF16, tag="wv", name="wv_slab")
        yt_slab = inpool.tile([P, NK * S], BF16, tag="yt", name="yt_slab")
        xt_slab = inpool.tile([P, NK * S], BF16, tag="xt", name="xt_slab")
        wq_slab = inpool.tile([P, 4 * NK * P], BF16, tag="wq", name="wq_slab")
        wk_slab = inpool.tile([P, 4 * NK * P], BF16, tag="wk", name="wk_slab")
        wo_slab = inpool.tile([P, 4 * HID], BF16, tag="wo", name="wo_slab")
        bias_sb = inpool.tile([P, NK], F32, tag="bias", name="bias_sb")
        vones_sb = inpool.tile([P, NHL], BF16, tag="vones", name="vones_sb")
        ident_sb = inpool.tile([P, P], BF16, tag="ident", name="ident_sb")

        # issue order on SP = earliest-needed first
        yt3 = yt_slab[:].rearrange("p (k c) -> p k c", c=S)
        xt3 = xt_slab[:].rearrange("p (k c) -> p k c", c=S)
        for q in range(8):
            if q >= 2 and q % 2 == 1:
                continue  # eighth-split only the first quarter
            span = 1 if q < 2 else 2
            wvs = slice(q * (NK * DQ // 8), (q + span) * (NK * DQ // 8))
            nc.sync.dma_start(wv_slab[:, wvs], wv[:, wvs])
            nc.sync.dma_start(
                yt3[:, q : q + span, 0:512],
                yt[:, q * (NK * 512 // 8) : (q + span) * (NK * 512 // 8)],
            )
        nc.sync.dma_start(yt3[:, :, 512:1024], yt[:, NK * 512 : 2 * NK * 512])
        nc.sync.dma_start(bias_sb[:], biasd[:])
        nc.sync.dma_start(vones_sb[:], onesd[:])
        nc.sync.dma_start(ident_sb[:], identd[:])
        sl0 = slice(0, NK * P)
        nc.sync.dma_start(wq_slab[:, sl0], wq[:, sl0])
        nc.sync.dma_start(wk_slab[:, sl0], wk[:, sl0])
        nc.sync.dma_start(xt3[:, :, 0:512], xt[:, 0 : NK * 512])
        nc.sync.dma_start(xt3[:, :, 512:1024], xt[:, NK * 512 : 2 * NK * 512])
        for pair in range(1, 4):
            sl = slice(pair * NK * P, (pair + 1) * NK * P)
            nc.sync.dma_start(wq_slab[:, sl], wq[:, sl])
            nc.sync.dma_start(wk_slab[:, sl], wk[:, sl])
        for pair in range(4):
            sl = slice(pair * HID, (pair + 1) * HID)
            nc.sync.dma_start(wo_slab[:, sl], wo[:, sl])

        def wv_k(k):
            return wv_slab[:, k * DQ : (k + 1) * DQ]

        def yt_k(k):
            return yt_slab[:, k * S : (k + 1) * S]

        def xt_k(k):
            return xt_slab[:, k * S : (k + 1) * S]

        def wqk_pk(slab, pair, k):
            base = pair * NK * P + k * P
            return slab[:, base : base + P]

        def wo_p(pair):
            return wo_slab[:, pair * HID : (pair + 1) * HID]

        # ---- persistent slabs ----
        qt_sb = [
            qkvpool.tile([P, S], BF16, tag=f"qt{m}", name=f"qt{m}") for m in range(4)
        ]
        kt_sb = [
            qkvpool.tile([P, S], BF16, tag=f"kt{m}", name=f"kt{m}") for m in range(4)
        ]
        v_sb = [
            qkvpool.tile([P, NHL * (DH + 1)], BF16, tag=f"v{m}", name=f"v{m}")
            for m in range(8)
        ]
        at_sb = [
            atpool.tile([P, S], BF16, tag=f"at{m}", name=f"at{m}") for m in range(4)
        ]
        acc_sb = [
            accpool.tile([P, HID], BF16, tag=f"acc{m}", name=f"acc{m}")
            for m in range(8)
        ]

        # PSUM (8 banks): GRAN=512 -> lg 2x[128,512]=2, av 2x[65,1024]=4,
        # sc 2x[128,512]=2; GRAN=1024 -> lg 2x[128,1024]=4, av 1x=2, sc 2.
        # (av first: the tail releases lg+sc but keeps av, so the tail pool
        # lands on banks whose last readers finished early)
        pp_av = tc.alloc_tile_pool(
            name="pp_av", bufs=2 if GRAN == 512 else 1, space="PSUM"
        )
        pp_lg = tc.alloc_tile_pool(name="pp_lg", bufs=2, space="PSUM")
        pp_sc = tc.alloc_tile_pool(name="pp_sc", bufs=2, space="PSUM")

        # ---- V projection (token-major, ones columns appended); emitted in
        # k-halves over m-pairs so matmuls trickle in as DMA quarters land ----
        for mp in range(0, 8, 2):
            pss = {}
            for kk in range(2):
                for m in (mp, mp + 1):
                    if kk == 0:
                        pss[m] = pp_sc.tile([P, DQ], F32, tag="sc", name="sc")
                    for k in range(kk * NK // 2, (kk + 1) * NK // 2):
                        nc.tensor.matmul(
                            pss[m][:],
                            yt_k(k)[:, m * P : (m + 1) * P],
                            wv_k(k),
                            start=(k == 0),
                            stop=(k == NK - 1),
                        )
            for m in (mp, mp + 1):
                dst3 = v_sb[m][:].rearrange("p (h c) -> p h c", c=DH + 1)
                src3 = pss[m][:].rearrange("p (h c) -> p h c", c=DH)
                nc.vector.tensor_copy(dst3[:, :, 0:DH], src3[:, :, :])
                nc.vector.tensor_copy(
                    dst3[:, :, DH : DH + 1],
                    vones_sb[:].rearrange("p (a b) -> p a b", b=1),
                )

        # ---- QT/KT projection for one pair as 32 emit-chunks of 1 matmul
        # (the last chunk of each psum tile appends the DVE eviction) ----
        def proj_chunks(pair):
            chunks = []
            for w_slab, src_k, dst in (
                (wq_slab, xt_k, qt_sb),
                (wk_slab, yt_k, kt_sb),
            ):
                for n in range(2):
                    ps_box = [None]

                    def mm(k, w_slab=w_slab, src_k=src_k, dst=dst, n=n, ps_box=ps_box):
                        if k == 0:
                            ps_box[0] = pp_sc.tile([P, 512], F32, tag="sc", name="sc")
                        nc.tensor.matmul(
                            ps_box[0][:],
                            wqk_pk(w_slab, pair, k),
                            src_k(k)[:, n * 512 : (n + 1) * 512],
                            start=(k == 0),
                            stop=(k == NK - 1),
                        )
                        if k == NK - 1:
                            nc.vector.tensor_copy(
                                dst[pair][:, n * 512 : (n + 1) * 512], ps_box[0][:]
                            )

                    for k in range(NK):
                        chunks.append(lambda k=k, mm=mm: mm(k))
            return chunks

        # ---- pairs 0-2 of the output projection: one PSUM accumulation
        # group per (m, n) tile, evicted to fp32 SBUF partials; two chunks
        # per tile ----
        def wo012_chunks():
            # last RESERVE tiles are held back to run during the final
            # head's normalize; their evictions go to the then-idle ACT
            chunks = []
            for m in range(8):
                for n in range(2):
                    ps_box = [None]
                    act_evict = 2 * m + n >= 16 - WO_RESERVE

                    def part1(m=m, n=n, ps_box=ps_box):
                        ps_box[0] = pp_sc.tile([P, 512], F32, tag="sc", name="sc")
                        for pair in range(2):
                            nc.tensor.matmul(
                                ps_box[0][:],
                                at_sb[pair][:, m * P : (m + 1) * P],
                                wo_p(pair)[:, n * 512 : (n + 1) * 512],
                                start=(pair == 0),
                                stop=False,
                            )

                    def part2(m=m, n=n, ps_box=ps_box, act_evict=act_evict):
                        nc.tensor.matmul(
                            ps_box[0][:],
                            at_sb[2][:, m * P : (m + 1) * P],
                            wo_p(2)[:, n * 512 : (n + 1) * 512],
                            start=False,
                            stop=True,
                        )
                        dst = acc_sb[m][:, n * 512 : (n + 1) * 512]
                        if act_evict:
                            nc.scalar.activation(dst, ps_box[0][:], COPY)
                        else:
                            nc.vector.tensor_copy(dst, ps_box[0][:])

                    chunks.append(part1)
                    chunks.append(part2)
            return chunks

        # ---- head section: one flat stream of 128 half-iterations
        # (head, sk, n). attn@V matmuls lag by LAG slots via a deferred
        # queue so they never make the PE wait on the exp stream. ----
        LAG = 3 if GRAN == 512 else 2
        WO_RESERVE = 2
        pending = {}
        gctr = [0]

        def emit_head(h, extras, delay=0):
            pair, hi = divmod(h, 2)
            base = hi * DH
            av = pp_av.tile([DH + 1, S], F32, tag="av", name="av")
            ei = 0
            nslots = 2 * NK if GRAN == 512 else NK
            for j in range(nslots):
                g = gctr[0]
                gctr[0] += 1
                if GRAN == 512:
                    sk, n = divmod(j, 2)
                    nhs = [n]
                else:
                    sk, nhs = j, [0, 1]
                lg = pp_lg.tile([P, GRAN], F32, tag="lg", name="lg")
                for li, n in enumerate(nhs):
                    nc.tensor.matmul(
                        lg[:, li * 512 : (li + 1) * 512],
                        kt_sb[pair][base : base + DH, sk * P : (sk + 1) * P],
                        qt_sb[pair][base : base + DH, n * 512 : (n + 1) * 512],
                        start=True,
                        stop=True,
                    )
                e = exppool.tile([P, GRAN], BF16, tag="exp", name="exp")
                nc.scalar.activation(e[:], lg[:], EXP, bias=bias_sb[:, sk : sk + 1])
                # lagged attn@V / normalize closures first: extras of the
                # next phase may read what the trailing normalizes write
                for fn in pending.pop(g, []):
                    fn()
                # interleaved PE work runs while ACT streams the exp
                if j >= delay:
                    take = (len(extras) - ei + (nslots - 1 - j)) // (nslots - j - (delay - j if j < delay else 0))
                    for _ in range(take):
                        extras[ei]()
                        ei += 1

                def av_mm(sk=sk, nhs=nhs, e=e, av=av, h=h):
                    for li, n in enumerate(nhs):
                        nc.tensor.matmul(
                            av[:, n * 512 : (n + 1) * 512],
                            v_sb[sk][:, h * (DH + 1) : (h + 1) * (DH + 1)],
                            e[:, li * 512 : (li + 1) * 512],
                            start=(sk == 0),
                            stop=(sk == NK - 1),
                        )

                pending.setdefault(g + LAG, []).append(av_mm)
                if sk == NK - 1:
                    # normalize per n-half right after the last attn@V
                    def norm(n, av=av, pair=pair, base=base):
                        cs = slice(n * 512, (n + 1) * 512)
                        rc = smallpool.tile([1, S], F32, tag="rc", name="rc")
                        nc.vector.reciprocal(rc[:, cs], av[DH : DH + 1, cs])
                        bc_sb = smallpool.tile([DH, S], F32, tag="bcsb", name="bcsb")
                        nc.gpsimd.partition_broadcast(bc_sb[:, cs], rc[:, cs])
                        nc.vector.tensor_mul(
                            at_sb[pair][base : base + DH, cs],
                            av[0:DH, cs],
                            bc_sb[:, cs],
                        )

                    for nn_ in nhs:
                        pending.setdefault(g + LAG, []).append(
                            lambda norm=norm, nn_=nn_: norm(n=nn_)
                        )
            assert ei == len(extras)

        # proj for pair 0 runs standalone (DMA-gated region anyway)
        for ch in proj_chunks(0):
            ch()
        # heads 0..5 carry the next pair's projections; 6..7 carry the
        # pair-0..2 output projection groups
        for pair in range(3):
            nxt = proj_chunks(pair + 1)
            emit_head(2 * pair, nxt[:16])
            emit_head(2 * pair + 1, nxt[16:])
        wo012 = wo012_chunks()
        nres = 2 * WO_RESERVE
        emit_head(6, wo012[:16], delay=3)
        emit_head(7, wo012[16 : 32 - nres])
        # flush trailing lagged attn@V + normalize closures, interleaving the
        # reserved wo012 chunks so the PE stays busy through the normalize
        reserved = wo012[32 - nres :]
        flush = []
        for g in sorted(pending.keys()):
            flush.extend(pending.pop(g))
        fi = ri = 0
        while fi < len(flush) or ri < len(reserved):
            if fi < len(flush):
                flush[fi]()
                fi += 1
            if ri < len(reserved):
                reserved[ri]()
                ri += 1

        # ---- tail: per m-tile PSUM group = identity matmul injecting the
        # bf16 pairs-0..2 partial (no at3 dependency -> runs during the last
        # normalize) + the pair-3 matmul; one ACT/DVE copy -> bf16 streams
        # out. No vector adds. ----
        pp_sc.release()
        pp_lg.release()
        pp_tail = tc.alloc_tile_pool(name="pp_tail", bufs=4, space="PSUM")
        WARM = 4
        units = [(m, n) for m in range(8) for n in range(2)]
        tail_ps = {}
        ob_tiles = {}

        def emit_ident(u):
            m, n = units[u]
            ps = pp_tail.tile([P, 512], F32, tag="tl", name="tl")
            tail_ps[u] = ps
            nc.tensor.matmul(
                ps[:],
                ident_sb[:],
                acc_sb[m][:, n * 512 : (n + 1) * 512],
                start=True,
                stop=False,
            )

        for u in range(WARM):
            emit_ident(u)
        for u in range(16):
            m, n = units[u]
            ps = tail_ps[u]
            nc.tensor.matmul(
                ps[:],
                at_sb[3][:, m * P : (m + 1) * P],
                wo_p(3)[:, n * 512 : (n + 1) * 512],
                start=False,
                stop=True,
            )
            if u + WARM < 16:
                emit_ident(u + WARM)
            if n == 0:
                ob_tiles[m] = outpool.tile([P, HID], BF16, tag="ob", name="ob")
            ob = ob_tiles[m]
            dst = ob[:, n * 512 : (n + 1) * 512]
            if u % 2 == 1:
                nc.vector.tensor_copy(dst, ps[:])
            else:
                nc.scalar.activation(dst, ps[:], COPY)
            if n == 1:
                nc.sync.dma_start(out[m * P : (m + 1) * P, :], ob[:])
        pp_tail.release()
        pp_av.release()


def _prep_in_maps(x, y, bias, Wq, Wk, Wv, Wo):
    x = np.asarray(x, dtype=np.float32)
    y = np.asarray(y, dtype=np.float32)
    bias = np.asarray(bias, dtype=np.float32)
    Wq = np.asarray(Wq, dtype=np.float32)
    Wk = np.asarray(Wk, dtype=np.float32)
    Wv = np.asarray(Wv, dtype=np.float32)
    Wo = np.asarray(Wo, dtype=np.float32)
    scale = 1.0 / np.sqrt(DH)
    dt = ml_dtypes.bfloat16

    def act_slab(a):
        # activation a [S, HID] -> slab halves layout [128, 2*NK*512]:
        # [:, h*NK*512 + k*512 + c] = a.T[k*128+p, h*512+c]
        at = a.T.reshape(NK, P, 2, 512)  # [k, p, h, c]
        return np.ascontiguousarray(at.transpose(1, 2, 0, 3).reshape(P, 2 * NK * 512))

    def w_pair_slab(w):
        # weights [1024, 512] -> pair-major slab [128, (pair k c128)]
        wr = w.reshape(NK, P, 4, P)  # [k, p, pair, c]
        return np.ascontiguousarray(wr.transpose(1, 2, 0, 3).reshape(P, 4 * NK * P))

    def wv_slab(w):
        # weights [1024, 512] -> k-major slab [128, (k c512)]
        wr = w.reshape(NK, P, DQ)
        return np.ascontiguousarray(wr.transpose(1, 0, 2).reshape(P, NK * DQ))

    def wo_slab(w):
        # [512, 1024] -> pair-major slab [128, (pair c1024)]
        wr = w.reshape(4, P, HID)
        return np.ascontiguousarray(wr.transpose(1, 0, 2).reshape(P, 4 * HID))

    in_maps = []
    for c in range(N_CORES):
        b, hf = divmod(c, 2)
        cols = slice(hf * DQ, (hf + 1) * DQ)
        in_maps.append(
            {
                "xt": act_slab(x[b]).astype(dt),
                "yt": act_slab(y[b]).astype(dt),
                "wq": w_pair_slab(Wq[:, cols] * scale).astype(dt),
                "wk": w_pair_slab(Wk[:, cols]).astype(dt),
                "wv": wv_slab(Wv[:, cols]).astype(dt),
                "wo": wo_slab(Wo[cols, :]).astype(dt),
                "biasd": np.ascontiguousarray(bias[b, 0, 0].reshape(NK, P).T),
                "onesd": np.ones((P, NHL), dtype=dt),
                "identd": np.eye(P, dtype=dt),
            }
        )
    return in_maps


def get_program():
    global _CACHED_NC
    if _CACHED_NC is None:
        _CACHED_NC = build_program()
    return _CACHED_NC


def kernel(x, y, bias, Wq, Wk, Wv, Wo):
    nc = get_program()
    in_maps = _prep_in_maps(x, y, bias, Wq, Wk, Wv, Wo)
    res = bass_utils.run_bass_kernel_spmd(nc, in_maps, core_ids=list(range(N_CORES)))
    B = 4
    out = np.empty((B, S, HID), dtype=np.float32)
    for b in range(B):
        out[b] = res.results[2 * b]["out"].astype(np.float32) + res.results[2 * b + 1][
            "out"
        ].astype(np.float32)
    return out
